# revision 5
# baseline (speedup 1.0000x reference)
"""MelSpectrogram Trainium2 kernel.

Full inputs in, full output out. Data-parallel over batch B=8 across the
8 NeuronCores (one audio row per core); DFT basis and mel filterbank are
replicated (prepped host-side into matmul-friendly layouts).

Per-core device algorithm (one audio row, T=1048576):
  The reflect-padded signal x (len 4099*256) is laid out host-side as two
  SBUF-resident tensors ste/sto [128, 4099] with ste[l,u] = x[u*256+l],
  sto[l,u] = x[u*256+128+l]. Because HOP=256 divides FILTER_LEN=1024, the
  windowed-DFT over frames is 8 shifted PSUM-accumulated matmuls per
  output tile: contraction pass p (taps [p*128,(p+1)*128)) uses
  rhs = st_{p%2}[:, f + p//2] -- no frame materialization, no 4x data blowup.
  Mel filter weights at bins 0 and 512 are exactly zero (fmin=0,
  fmax=sr/2 edge filters), so only bins 0..511 are computed: clean 4x128
  partition tiles, no ragged 513th row.
  magnitude = sqrt(R^2+I^2) on ACT+DVE, mel projection on PE, final
  log1p(1e4*x) = Ln(1e4*x + 1) on ACT.
"""

import os
import sys

sys.path.insert(0, "/opt/trn_rl_repo")

import numpy as np
import concourse.bass as bass
import concourse.mybir as mybir
import concourse.tile as tile
from concourse.bass_utils import run_bass_kernel_spmd
from concourse.vector_clock import ScopedClock

N_CORES = 8
T = 1048576
PAD = 384
SEG = 4099  # (T + 2*PAD) / 256
F = 4096  # output frames
NT = 512  # frames per tile
N_TILES = F // NT
DT = mybir.dt.float16
NP_DT = np.float16

_cache = {}


class _PatchedTileContext(tile.TileContext):
    # This walrus build rejects >1 sync-wait on the kernel-tail Drain
    # (CoreV3 NO_STRUCT template): carry the waits on one-NoOp-per-wait
    # ahead of the drain instead.
    def _drain_and_barrier(self, tick_clock, wait_clock):
        nop_inst = self.nc.sync.nop(nofuse=True, hint="pre_drain_waits")
        wait_clock.add_sem_waits(
            nop_inst.ins, ScopedClock({None: tick_clock.global_clock})
        )
        waits = list(nop_inst.ins.sync_info.on_wait)
        if len(waits) > 1:
            si = nop_inst.ins.sync_info
            si.on_wait = waits[:1]
            nop_inst.ins.sync_info = si
            for w in waits[1:]:
                extra = self.nc.sync.nop(nofuse=True, hint="pre_drain_waits")
                esi = extra.ins.sync_info or mybir.SyncInfo(on_wait=[], on_update=[])
                esi.on_wait = [w]
                extra.ins.sync_info = esi
        self.nc.sync.drain()
        self.nc.all_engine_barrier()
        assert self.sems is not None
        popped = self.nc._tile_sem_poison_stack.pop()
        assert popped is self._sem_poison
        self.nc.clear_and_free_semaphores(list(self.sems.allocated().values()))
        self.nc.all_engine_barrier()


def _split_sync_waits(nc, cap=1):
    # This walrus build encodes at most one sync-wait per instruction.
    # Hoist excess waits onto same-engine NoOps placed just before the
    # instruction (engines are in-order, so this preserves semantics).
    for f in nc.m.functions:
        for bb in f.blocks:
            out = []
            changed = False
            for inst in bb.instructions:
                si = inst.sync_info
                waits = list(si.on_wait) if si else []
                if len(waits) > cap:
                    changed = True
                    for w in waits[:-cap]:
                        nop = mybir.InstNoOp(
                            name=nc.get_next_instruction_name(), ins=[], outs=[]
                        )
                        nop.engine = inst.engine
                        nop.sync_info = mybir.SyncInfo(on_wait=[w], on_update=[])
                        out.append(nop)
                    si.on_wait = waits[-cap:]
                    inst.sync_info = si
                out.append(inst)
            if changed:
                bb.instructions = out


def _build_program():
    nc = bass.Bass()
    ste_d = nc.dram_tensor("ste", [128, SEG], DT, kind="ExternalInput")
    sto_d = nc.dram_tensor("sto", [128, SEG], DT, kind="ExternalInput")
    wts_d = nc.dram_tensor("wts", [128, 64 * 128], DT, kind="ExternalInput")
    melt_d = nc.dram_tensor("melt", [128, 4 * 80], DT, kind="ExternalInput")
    out_d = nc.dram_tensor("out", [80, F], mybir.dt.float32, kind="ExternalOutput")

    f32 = mybir.dt.float32
    with _PatchedTileContext(nc) as tc:
        with (
            tc.tile_pool(name="const", bufs=1) as const,
            tc.tile_pool(name="work", bufs=2) as work,
            tc.tile_pool(name="psum", bufs=2, space="PSUM") as pp,
        ):
            w_sb = const.tile([128, 64 * 128], DT)
            nc.sync.dma_start(out=w_sb[:], in_=wts_d[:])
            melt_sb = const.tile([128, 4 * 80], DT)
            nc.sync.dma_start(out=melt_sb[:], in_=melt_d[:])
            ste_sb = const.tile([128, SEG], DT)
            nc.sync.dma_start(out=ste_sb[:], in_=ste_d[:])
            sto_sb = const.tile([128, SEG], DT)
            nc.sync.dma_start(out=sto_sb[:], in_=sto_d[:])
            outbuf = const.tile([80, F], f32)

            for t in range(N_TILES):
                t0 = t * NT
                mags = []
                for m in range(4):
                    rps = pp.tile([128, NT], f32, tag="R")
                    ips = pp.tile([128, NT], f32, tag="I")
                    for p in range(8):
                        st = ste_sb if p % 2 == 0 else sto_sb
                        rhs = st[:, t0 + p // 2 : t0 + p // 2 + NT]
                        wq = ((p * 4 + m) * 2) * 128
                        nc.tensor.matmul(
                            rps[:], w_sb[:, wq : wq + 128], rhs,
                            start=(p == 0), stop=(p == 7),
                        )
                    for p in range(8):
                        st = ste_sb if p % 2 == 0 else sto_sb
                        rhs = st[:, t0 + p // 2 : t0 + p // 2 + NT]
                        wq = ((p * 4 + m) * 2 + 1) * 128
                        nc.tensor.matmul(
                            ips[:], w_sb[:, wq : wq + 128], rhs,
                            start=(p == 0), stop=(p == 7),
                        )
                    a = work.tile([128, NT], f32, tag="a")
                    nc.scalar.square(a[:], rps[:])
                    b = work.tile([128, NT], f32, tag="b")
                    nc.scalar.square(b[:], ips[:])
                    m2 = work.tile([128, NT], f32, tag="m2")
                    nc.vector.tensor_tensor(m2[:], a[:], b[:], mybir.AluOpType.add)
                    mg = work.tile([128, NT], DT, tag=f"mag{m}")
                    nc.scalar.sqrt(mg[:], m2[:])
                    mags.append(mg)
                mel_ps = pp.tile([80, NT], f32, tag="mel")
                for m in range(4):
                    nc.tensor.matmul(
                        mel_ps[:], melt_sb[:, m * 80 : (m + 1) * 80], mags[m][:],
                        start=(m == 0), stop=(m == 3),
                    )
                nc.vector.tensor_copy(out=outbuf[:, t0 : t0 + NT], in_=mel_ps[:])

            outln = const.tile([80, F], f32)
            nc.scalar.activation(
                outln[:], outbuf[:], mybir.ActivationFunctionType.Ln,
                bias=1.0, scale=10000.0,
            )
            nc.sync.dma_start(out=out_d[:], in_=outln[:])
    _split_sync_waits(nc)
    return nc


def _prep_inputs(audio, basis_r, basis_i, mel_basis):
    audio = np.asarray(audio, dtype=np.float32)
    basis_r = np.asarray(basis_r, dtype=np.float32)
    basis_i = np.asarray(basis_i, dtype=np.float32)
    mel_basis = np.asarray(mel_basis, dtype=np.float32)

    # wts[:, ((p*4+m)*2+s)*128 + j] = basis_s[m*128+j, p*128:(p+1)*128]
    wts = np.empty((128, 64 * 128), dtype=NP_DT)
    for p in range(8):
        for m in range(4):
            for s, basis in enumerate((basis_r, basis_i)):
                q = ((p * 4 + m) * 2 + s) * 128
                wts[:, q : q + 128] = basis[
                    m * 128 : (m + 1) * 128, p * 128 : (p + 1) * 128
                ].T.astype(NP_DT)
    melt = np.empty((128, 4 * 80), dtype=NP_DT)
    for m in range(4):
        melt[:, m * 80 : (m + 1) * 80] = mel_basis[
            :, m * 128 : (m + 1) * 128
        ].T.astype(NP_DT)

    in_maps = []
    for b in range(N_CORES):
        row = audio[b]
        x = np.concatenate([row[PAD:0:-1], row, row[-2 : -PAD - 2 : -1]])
        xr = x.reshape(SEG, 256)
        ste = np.ascontiguousarray(xr[:, :128].T).astype(NP_DT)
        sto = np.ascontiguousarray(xr[:, 128:].T).astype(NP_DT)
        in_maps.append({"ste": ste, "sto": sto, "wts": wts, "melt": melt})
    return in_maps


def kernel(audio, basis_r, basis_i, mel_basis):
    if "nc" not in _cache:
        _cache["nc"] = _build_program()
    nc = _cache["nc"]
    in_maps = _prep_inputs(audio, basis_r, basis_i, mel_basis)

    trace = os.environ.get("MELSPEC_TRACE") == "1"
    if trace:
        import types
        import trn_agent_boot.trn_boot as tb
        import concourse.bass_utils as bu

        if "antenv.axon_hooks" not in sys.modules:
            hook = tb._ntff_profile_via_ctypes("/opt/axon/libaxon_pjrt.so")
            mod = types.ModuleType("antenv.axon_hooks")
            mod.get_axon_ntff_profile_hook = lambda: hook
            sys.modules["antenv.axon_hooks"] = mod
        bu.upload_artifacts = lambda tmpdir: f"local://{tmpdir}"

    res = run_bass_kernel_spmd(nc, in_maps, list(range(N_CORES)), trace=trace)
    _cache["last_results"] = res
    out = np.stack([res.results[i]["out"] for i in range(N_CORES)])
    return out.astype(np.float32)


# revision 9
# speedup vs baseline: 1.0077x; 1.0077x over previous
"""MelSpectrogram Trainium2 kernel.

Full inputs in, full output out. Data-parallel over batch B=8 across the
8 NeuronCores (one audio row per core); DFT basis and mel filterbank are
replicated (prepped host-side into matmul-friendly layouts).

Per-core device algorithm (one audio row, T=1048576):
  The reflect-padded signal x (len 4099*256) is laid out host-side as two
  SBUF-resident tensors ste/sto [128, 4099] with ste[l,u] = x[u*256+l],
  sto[l,u] = x[u*256+128+l]. Because HOP=256 divides FILTER_LEN=1024, the
  windowed-DFT over frames is 8 shifted PSUM-accumulated matmuls per
  output tile: contraction pass p (taps [p*128,(p+1)*128)) uses
  rhs = st_{p%2}[:, f + p//2] -- no frame materialization, no 4x data blowup.
  Mel filter weights at bins 0 and 512 are exactly zero (fmin=0,
  fmax=sr/2 edge filters), so only bins 0..511 are computed: clean 4x128
  partition tiles, no ragged 513th row.
  magnitude = sqrt(R^2+I^2) on ACT+DVE, mel projection on PE, final
  log1p(1e4*x) = Ln(1e4*x + 1) on ACT.
"""

import os
import sys

sys.path.insert(0, "/opt/trn_rl_repo")

import numpy as np
import concourse.bass as bass
import concourse.mybir as mybir
import concourse.tile as tile
from concourse.bass_utils import run_bass_kernel_spmd
from concourse.vector_clock import ScopedClock

N_CORES = 8
T = 1048576
PAD = 384
SEG = 4099  # (T + 2*PAD) / 256
F = 4096  # output frames
NT = 512  # frames per tile
N_TILES = F // NT
DT = mybir.dt.float16
NP_DT = np.float16

_cache = {}


class _PatchedTileContext(tile.TileContext):
    # This walrus build rejects >1 sync-wait on the kernel-tail Drain
    # (CoreV3 NO_STRUCT template): carry the waits on one-NoOp-per-wait
    # ahead of the drain instead.
    def _drain_and_barrier(self, tick_clock, wait_clock):
        nop_inst = self.nc.sync.nop(nofuse=True, hint="pre_drain_waits")
        wait_clock.add_sem_waits(
            nop_inst.ins, ScopedClock({None: tick_clock.global_clock})
        )
        waits = list(nop_inst.ins.sync_info.on_wait)
        if len(waits) > 1:
            si = nop_inst.ins.sync_info
            si.on_wait = waits[:1]
            nop_inst.ins.sync_info = si
            for w in waits[1:]:
                extra = self.nc.sync.nop(nofuse=True, hint="pre_drain_waits")
                esi = extra.ins.sync_info or mybir.SyncInfo(on_wait=[], on_update=[])
                esi.on_wait = [w]
                extra.ins.sync_info = esi
        self.nc.sync.drain()
        self.nc.all_engine_barrier()
        assert self.sems is not None
        popped = self.nc._tile_sem_poison_stack.pop()
        assert popped is self._sem_poison
        self.nc.clear_and_free_semaphores(list(self.sems.allocated().values()))
        self.nc.all_engine_barrier()


def _split_sync_waits(nc, cap=1):
    # This walrus build encodes at most one sync-wait per instruction.
    # Hoist excess waits onto same-engine NoOps placed just before the
    # instruction (engines are in-order, so this preserves semantics).
    for f in nc.m.functions:
        for bb in f.blocks:
            out = []
            changed = False
            for inst in bb.instructions:
                si = inst.sync_info
                waits = list(si.on_wait) if si else []
                if len(waits) > cap:
                    changed = True
                    for w in waits[:-cap]:
                        nop = mybir.InstNoOp(
                            name=nc.get_next_instruction_name(), ins=[], outs=[]
                        )
                        nop.engine = inst.engine
                        nop.sync_info = mybir.SyncInfo(on_wait=[w], on_update=[])
                        out.append(nop)
                    si.on_wait = waits[-cap:]
                    inst.sync_info = si
                out.append(inst)
            if changed:
                bb.instructions = out


def _build_program():
    nc = bass.Bass()
    ste_d = nc.dram_tensor("ste", [128, SEG], DT, kind="ExternalInput")
    sto_d = nc.dram_tensor("sto", [128, SEG], DT, kind="ExternalInput")
    wts_d = nc.dram_tensor("wts", [128, 64 * 128], DT, kind="ExternalInput")
    melt_d = nc.dram_tensor("melt", [128, 4 * 80], DT, kind="ExternalInput")
    out_d = nc.dram_tensor("out", [80, F], mybir.dt.float32, kind="ExternalOutput")

    f32 = mybir.dt.float32
    with _PatchedTileContext(nc) as tc:
        with (
            tc.tile_pool(name="const", bufs=1) as const,
            tc.tile_pool(name="work", bufs=2) as work,
            tc.tile_pool(name="psum", bufs=2, space="PSUM") as pp,
        ):
            # weights grouped by m: 4 chunks so the first matmuls are
            # gated on 512KB, not the full 2MB
            w_sb = const.tile([128, 64 * 128], DT)
            for m in range(4):
                nc.sync.dma_start(
                    out=w_sb[:, m * 2048 : (m + 1) * 2048],
                    in_=wts_d[:, m * 2048 : (m + 1) * 2048],
                )
            melt_sb = const.tile([128, 4 * 80], DT)
            nc.sync.dma_start(out=melt_sb[:], in_=melt_d[:])
            # audio chunked per N-tile (+3 overlap columns), separate tiles
            # so tile t's matmuls only gate on chunk t's DMA
            ste_c, sto_c = [], []
            for t in range(N_TILES):
                c0 = t * NT
                c1 = min(SEG, (t + 1) * NT + 3)
                se = const.tile([128, c1 - c0], DT, tag=f"ste{t}")
                nc.sync.dma_start(out=se[:], in_=ste_d[:, c0:c1])
                so = const.tile([128, c1 - c0], DT, tag=f"sto{t}")
                nc.sync.dma_start(out=so[:], in_=sto_d[:, c0:c1])
                ste_c.append(se)
                sto_c.append(so)
            outbuf = const.tile([80, F], f32)

            for t in range(N_TILES):
                t0 = t * NT
                mags = []
                for m in range(4):
                    rps = pp.tile([128, NT], f32, tag="R")
                    ips = pp.tile([128, NT], f32, tag="I")
                    for p in range(8):
                        st = ste_c[t] if p % 2 == 0 else sto_c[t]
                        rhs = st[:, p // 2 : p // 2 + NT]
                        wq = ((m * 8 + p) * 2) * 128
                        nc.tensor.matmul(
                            rps[:], w_sb[:, wq : wq + 128], rhs,
                            start=(p == 0), stop=(p == 7),
                        )
                    for p in range(8):
                        st = ste_c[t] if p % 2 == 0 else sto_c[t]
                        rhs = st[:, p // 2 : p // 2 + NT]
                        wq = ((m * 8 + p) * 2 + 1) * 128
                        nc.tensor.matmul(
                            ips[:], w_sb[:, wq : wq + 128], rhs,
                            start=(p == 0), stop=(p == 7),
                        )
                    a = work.tile([128, NT], f32, tag="a")
                    nc.scalar.square(a[:], rps[:])
                    b = work.tile([128, NT], f32, tag="b")
                    nc.scalar.square(b[:], ips[:])
                    m2 = work.tile([128, NT], f32, tag="m2")
                    nc.vector.tensor_tensor(m2[:], a[:], b[:], mybir.AluOpType.add)
                    mg = work.tile([128, NT], DT, tag=f"mag{m}")
                    nc.scalar.sqrt(mg[:], m2[:])
                    mags.append(mg)
                mel_ps = pp.tile([80, NT], f32, tag="mel")
                for m in range(4):
                    nc.tensor.matmul(
                        mel_ps[:], melt_sb[:, m * 80 : (m + 1) * 80], mags[m][:],
                        start=(m == 0), stop=(m == 3),
                    )
                nc.vector.tensor_copy(out=outbuf[:, t0 : t0 + NT], in_=mel_ps[:])

            outln = const.tile([80, F], f32)
            nc.scalar.activation(
                outln[:], outbuf[:], mybir.ActivationFunctionType.Ln,
                bias=1.0, scale=10000.0,
            )
            nc.sync.dma_start(out=out_d[:], in_=outln[:])
    _split_sync_waits(nc)
    return nc


def _prep_inputs(audio, basis_r, basis_i, mel_basis):
    audio = np.asarray(audio, dtype=np.float32)
    basis_r = np.asarray(basis_r, dtype=np.float32)
    basis_i = np.asarray(basis_i, dtype=np.float32)
    mel_basis = np.asarray(mel_basis, dtype=np.float32)

    # wts[:, ((m*8+p)*2+s)*128 + j] = basis_s[m*128+j, p*128:(p+1)*128]
    wts = np.empty((128, 64 * 128), dtype=NP_DT)
    for p in range(8):
        for m in range(4):
            for s, basis in enumerate((basis_r, basis_i)):
                q = ((m * 8 + p) * 2 + s) * 128
                wts[:, q : q + 128] = basis[
                    m * 128 : (m + 1) * 128, p * 128 : (p + 1) * 128
                ].T.astype(NP_DT)
    melt = np.empty((128, 4 * 80), dtype=NP_DT)
    for m in range(4):
        melt[:, m * 80 : (m + 1) * 80] = mel_basis[
            :, m * 128 : (m + 1) * 128
        ].T.astype(NP_DT)

    in_maps = []
    for b in range(N_CORES):
        row = audio[b]
        x = np.concatenate([row[PAD:0:-1], row, row[-2 : -PAD - 2 : -1]])
        xr = x.reshape(SEG, 256)
        ste = np.ascontiguousarray(xr[:, :128].T).astype(NP_DT)
        sto = np.ascontiguousarray(xr[:, 128:].T).astype(NP_DT)
        in_maps.append({"ste": ste, "sto": sto, "wts": wts, "melt": melt})
    return in_maps


def kernel(audio, basis_r, basis_i, mel_basis):
    if "nc" not in _cache:
        _cache["nc"] = _build_program()
    nc = _cache["nc"]
    in_maps = _prep_inputs(audio, basis_r, basis_i, mel_basis)

    trace = os.environ.get("MELSPEC_TRACE") == "1"
    if trace:
        import types
        import trn_agent_boot.trn_boot as tb
        import concourse.bass_utils as bu

        if "antenv.axon_hooks" not in sys.modules:
            hook = tb._ntff_profile_via_ctypes("/opt/axon/libaxon_pjrt.so")
            mod = types.ModuleType("antenv.axon_hooks")
            mod.get_axon_ntff_profile_hook = lambda: hook
            sys.modules["antenv.axon_hooks"] = mod
        bu.upload_artifacts = lambda tmpdir: f"local://{tmpdir}"

    res = run_bass_kernel_spmd(nc, in_maps, list(range(N_CORES)), trace=trace)
    _cache["last_results"] = res
    out = np.stack([res.results[i]["out"] for i in range(N_CORES)])
    return out.astype(np.float32)


# revision 11
# speedup vs baseline: 1.0314x; 1.0236x over previous
"""MelSpectrogram Trainium2 kernel.

Full inputs in, full output out. Data-parallel over batch B=8 across the
8 NeuronCores (one audio row per core); DFT basis and mel filterbank are
replicated (prepped host-side into matmul-friendly layouts).

Per-core device algorithm (one audio row, T=1048576):
  The reflect-padded signal x (len 4099*256) is laid out host-side as two
  SBUF-resident tensors ste/sto [128, 4099] with ste[l,u] = x[u*256+l],
  sto[l,u] = x[u*256+128+l]. Because HOP=256 divides FILTER_LEN=1024, the
  windowed-DFT over frames is 8 shifted PSUM-accumulated matmuls per
  output tile: contraction pass p (taps [p*128,(p+1)*128)) uses
  rhs = st_{p%2}[:, f + p//2] -- no frame materialization, no 4x data blowup.
  Mel filter weights at bins 0 and 512 are exactly zero (fmin=0,
  fmax=sr/2 edge filters), so only bins 0..511 are computed: clean 4x128
  partition tiles, no ragged 513th row.
  magnitude = sqrt(R^2+I^2) on ACT+DVE, mel projection on PE, final
  log1p(1e4*x) = Ln(1e4*x + 1) on ACT.
"""

import os
import sys

sys.path.insert(0, "/opt/trn_rl_repo")

import numpy as np
import concourse.bass as bass
import concourse.mybir as mybir
import concourse.tile as tile
from concourse.bass_utils import run_bass_kernel_spmd
from concourse.vector_clock import ScopedClock

N_CORES = 8
T = 1048576
PAD = 384
SEG = 4099  # (T + 2*PAD) / 256
F = 4096  # output frames
NT = 512  # frames per tile
N_TILES = F // NT
DT = mybir.dt.float16
NP_DT = np.float16

_cache = {}


class _PatchedTileContext(tile.TileContext):
    # This walrus build rejects >1 sync-wait on the kernel-tail Drain
    # (CoreV3 NO_STRUCT template): carry the waits on one-NoOp-per-wait
    # ahead of the drain instead.
    def _drain_and_barrier(self, tick_clock, wait_clock):
        nop_inst = self.nc.sync.nop(nofuse=True, hint="pre_drain_waits")
        wait_clock.add_sem_waits(
            nop_inst.ins, ScopedClock({None: tick_clock.global_clock})
        )
        waits = list(nop_inst.ins.sync_info.on_wait)
        if len(waits) > 1:
            si = nop_inst.ins.sync_info
            si.on_wait = waits[:1]
            nop_inst.ins.sync_info = si
            for w in waits[1:]:
                extra = self.nc.sync.nop(nofuse=True, hint="pre_drain_waits")
                esi = extra.ins.sync_info or mybir.SyncInfo(on_wait=[], on_update=[])
                esi.on_wait = [w]
                extra.ins.sync_info = esi
        self.nc.sync.drain()
        self.nc.all_engine_barrier()
        assert self.sems is not None
        popped = self.nc._tile_sem_poison_stack.pop()
        assert popped is self._sem_poison
        self.nc.clear_and_free_semaphores(list(self.sems.allocated().values()))
        self.nc.all_engine_barrier()


def _split_sync_waits(nc, cap=1):
    # This walrus build encodes at most one sync-wait per instruction.
    # Hoist excess waits onto same-engine NoOps placed just before the
    # instruction (engines are in-order, so this preserves semantics).
    for f in nc.m.functions:
        for bb in f.blocks:
            out = []
            changed = False
            for inst in bb.instructions:
                si = inst.sync_info
                waits = list(si.on_wait) if si else []
                if len(waits) > cap:
                    changed = True
                    for w in waits[:-cap]:
                        nop = mybir.InstNoOp(
                            name=nc.get_next_instruction_name(), ins=[], outs=[]
                        )
                        nop.engine = inst.engine
                        nop.sync_info = mybir.SyncInfo(on_wait=[w], on_update=[])
                        out.append(nop)
                    si.on_wait = waits[-cap:]
                    inst.sync_info = si
                out.append(inst)
            if changed:
                bb.instructions = out


def _build_program():
    nc = bass.Bass()
    ste_d = nc.dram_tensor("ste", [128, SEG], DT, kind="ExternalInput")
    sto_d = nc.dram_tensor("sto", [128, SEG], DT, kind="ExternalInput")
    wts_d = nc.dram_tensor("wts", [128, 64 * 128], DT, kind="ExternalInput")
    melt_d = nc.dram_tensor("melt", [128, 4 * 80], DT, kind="ExternalInput")
    out_d = nc.dram_tensor("out", [80, F], mybir.dt.float32, kind="ExternalOutput")

    f32 = mybir.dt.float32
    with _PatchedTileContext(nc) as tc:
        with (
            tc.tile_pool(name="const", bufs=1) as const,
            tc.tile_pool(name="work", bufs=2) as work,
            tc.tile_pool(name="psum", bufs=2, space="PSUM") as pp,
        ):
            # HWDGE queues are FIFO and fair-share bandwidth, so emission
            # order + fine splitting of the first-tile-critical transfers
            # approximates a priority: m=0 weights (16x32KB), chunk-0
            # audio, then everything else.
            w_sb = const.tile([128, 64 * 128], DT)
            for q in range(16):
                nc.sync.dma_start(
                    out=w_sb[:, q * 128 : (q + 1) * 128],
                    in_=wts_d[:, q * 128 : (q + 1) * 128],
                )
            ste_c, sto_c = [], []
            for t in range(N_TILES):
                c0 = t * NT
                c1 = min(SEG, (t + 1) * NT + 3)
                se = const.tile([128, c1 - c0], DT, tag=f"ste{t}")
                so = const.tile([128, c1 - c0], DT, tag=f"sto{t}")
                ste_c.append(se)
                sto_c.append(so)
            nc.sync.dma_start(out=ste_c[0][:], in_=ste_d[:, 0 : NT + 3])
            nc.sync.dma_start(out=sto_c[0][:], in_=sto_d[:, 0 : NT + 3])
            melt_sb = const.tile([128, 4 * 80], DT)
            nc.sync.dma_start(out=melt_sb[:], in_=melt_d[:])
            for m in range(1, 4):
                nc.sync.dma_start(
                    out=w_sb[:, m * 2048 : (m + 1) * 2048],
                    in_=wts_d[:, m * 2048 : (m + 1) * 2048],
                )
            for t in range(1, N_TILES):
                c0 = t * NT
                c1 = min(SEG, (t + 1) * NT + 3)
                nc.sync.dma_start(out=ste_c[t][:], in_=ste_d[:, c0:c1])
                nc.sync.dma_start(out=sto_c[t][:], in_=sto_d[:, c0:c1])
            outbuf = const.tile([80, F], f32)

            for t in range(N_TILES):
                t0 = t * NT
                mags = []
                for m in range(4):
                    rps = pp.tile([128, NT], f32, tag="R")
                    ips = pp.tile([128, NT], f32, tag="I")
                    for p in range(8):
                        st = ste_c[t] if p % 2 == 0 else sto_c[t]
                        rhs = st[:, p // 2 : p // 2 + NT]
                        wq = ((m * 8 + p) * 2) * 128
                        nc.tensor.matmul(
                            rps[:], w_sb[:, wq : wq + 128], rhs,
                            start=(p == 0), stop=(p == 7),
                        )
                    for p in range(8):
                        st = ste_c[t] if p % 2 == 0 else sto_c[t]
                        rhs = st[:, p // 2 : p // 2 + NT]
                        wq = ((m * 8 + p) * 2 + 1) * 128
                        nc.tensor.matmul(
                            ips[:], w_sb[:, wq : wq + 128], rhs,
                            start=(p == 0), stop=(p == 7),
                        )
                    a = work.tile([128, NT], f32, tag="a")
                    nc.scalar.square(a[:], rps[:])
                    b = work.tile([128, NT], f32, tag="b")
                    nc.scalar.square(b[:], ips[:])
                    m2 = work.tile([128, NT], f32, tag="m2")
                    nc.vector.tensor_tensor(m2[:], a[:], b[:], mybir.AluOpType.add)
                    mg = work.tile([128, NT], DT, tag=f"mag{m}")
                    nc.scalar.sqrt(mg[:], m2[:])
                    mags.append(mg)
                mel_ps = pp.tile([80, NT], f32, tag="mel")
                for m in range(4):
                    nc.tensor.matmul(
                        mel_ps[:], melt_sb[:, m * 80 : (m + 1) * 80], mags[m][:],
                        start=(m == 0), stop=(m == 3),
                    )
                nc.vector.tensor_copy(out=outbuf[:, t0 : t0 + NT], in_=mel_ps[:])

            # stream the log1p + output DMA in 4 column blocks so the
            # store overlaps the activation
            outln = const.tile([80, F], f32)
            for b0 in range(0, F, 1024):
                nc.scalar.activation(
                    outln[:, b0 : b0 + 1024], outbuf[:, b0 : b0 + 1024],
                    mybir.ActivationFunctionType.Ln, bias=1.0, scale=10000.0,
                )
                nc.sync.dma_start(
                    out=out_d[:, b0 : b0 + 1024], in_=outln[:, b0 : b0 + 1024]
                )
    _split_sync_waits(nc)
    return nc


def _prep_inputs(audio, basis_r, basis_i, mel_basis):
    audio = np.asarray(audio, dtype=np.float32)
    basis_r = np.asarray(basis_r, dtype=np.float32)
    basis_i = np.asarray(basis_i, dtype=np.float32)
    mel_basis = np.asarray(mel_basis, dtype=np.float32)

    # wts[:, ((m*8+p)*2+s)*128 + j] = basis_s[m*128+j, p*128:(p+1)*128]
    wts = np.empty((128, 64 * 128), dtype=NP_DT)
    for p in range(8):
        for m in range(4):
            for s, basis in enumerate((basis_r, basis_i)):
                q = ((m * 8 + p) * 2 + s) * 128
                wts[:, q : q + 128] = basis[
                    m * 128 : (m + 1) * 128, p * 128 : (p + 1) * 128
                ].T.astype(NP_DT)
    melt = np.empty((128, 4 * 80), dtype=NP_DT)
    for m in range(4):
        melt[:, m * 80 : (m + 1) * 80] = mel_basis[
            :, m * 128 : (m + 1) * 128
        ].T.astype(NP_DT)

    in_maps = []
    for b in range(N_CORES):
        row = audio[b]
        x = np.concatenate([row[PAD:0:-1], row, row[-2 : -PAD - 2 : -1]])
        xr = x.reshape(SEG, 256)
        ste = np.ascontiguousarray(xr[:, :128].T).astype(NP_DT)
        sto = np.ascontiguousarray(xr[:, 128:].T).astype(NP_DT)
        in_maps.append({"ste": ste, "sto": sto, "wts": wts, "melt": melt})
    return in_maps


def kernel(audio, basis_r, basis_i, mel_basis):
    if "nc" not in _cache:
        _cache["nc"] = _build_program()
    nc = _cache["nc"]
    in_maps = _prep_inputs(audio, basis_r, basis_i, mel_basis)

    trace = os.environ.get("MELSPEC_TRACE") == "1"
    if trace:
        import types
        import trn_agent_boot.trn_boot as tb
        import concourse.bass_utils as bu

        if "antenv.axon_hooks" not in sys.modules:
            hook = tb._ntff_profile_via_ctypes("/opt/axon/libaxon_pjrt.so")
            mod = types.ModuleType("antenv.axon_hooks")
            mod.get_axon_ntff_profile_hook = lambda: hook
            sys.modules["antenv.axon_hooks"] = mod
        bu.upload_artifacts = lambda tmpdir: f"local://{tmpdir}"

    res = run_bass_kernel_spmd(nc, in_maps, list(range(N_CORES)), trace=trace)
    _cache["last_results"] = res
    out = np.stack([res.results[i]["out"] for i in range(N_CORES)])
    return out.astype(np.float32)


# revision 14
# speedup vs baseline: 1.0493x; 1.0173x over previous
"""MelSpectrogram Trainium2 kernel.

Full inputs in, full output out. Data-parallel over batch B=8 across the
8 NeuronCores (one audio row per core); DFT basis and mel filterbank are
replicated (prepped host-side into matmul-friendly layouts).

Per-core device algorithm (one audio row, T=1048576):
  The reflect-padded signal x (len 4099*256) is laid out host-side as two
  SBUF-resident tensors ste/sto [128, 4099] with ste[l,u] = x[u*256+l],
  sto[l,u] = x[u*256+128+l]. Because HOP=256 divides FILTER_LEN=1024, the
  windowed-DFT over frames is 8 shifted PSUM-accumulated matmuls per
  output tile: contraction pass p (taps [p*128,(p+1)*128)) uses
  rhs = st_{p%2}[:, f + p//2] -- no frame materialization, no 4x data blowup.
  Mel filter weights at bins 0 and 512 are exactly zero (fmin=0,
  fmax=sr/2 edge filters), so only bins 0..511 are computed: clean 4x128
  partition tiles, no ragged 513th row.
  magnitude = sqrt(R^2+I^2) on ACT+DVE, mel projection on PE, final
  log1p(1e4*x) = Ln(1e4*x + 1) on ACT.
"""

import os
import sys

sys.path.insert(0, "/opt/trn_rl_repo")

import numpy as np
import concourse.bass as bass
import concourse.mybir as mybir
import concourse.tile as tile
from concourse.bass_utils import run_bass_kernel_spmd
from concourse.vector_clock import ScopedClock

N_CORES = 8
T = 1048576
PAD = 384
SEG = 4099  # (T + 2*PAD) / 256
F = 4096  # output frames
NT = 512  # frames per tile
N_TILES = F // NT
DT = mybir.dt.float16
NP_DT = np.float16

_cache = {}


class _PatchedTileContext(tile.TileContext):
    # This walrus build rejects >1 sync-wait on the kernel-tail Drain
    # (CoreV3 NO_STRUCT template): carry the waits on one-NoOp-per-wait
    # ahead of the drain instead.
    def _drain_and_barrier(self, tick_clock, wait_clock):
        nop_inst = self.nc.sync.nop(nofuse=True, hint="pre_drain_waits")
        wait_clock.add_sem_waits(
            nop_inst.ins, ScopedClock({None: tick_clock.global_clock})
        )
        waits = list(nop_inst.ins.sync_info.on_wait)
        if len(waits) > 1:
            si = nop_inst.ins.sync_info
            si.on_wait = waits[:1]
            nop_inst.ins.sync_info = si
            for w in waits[1:]:
                extra = self.nc.sync.nop(nofuse=True, hint="pre_drain_waits")
                esi = extra.ins.sync_info or mybir.SyncInfo(on_wait=[], on_update=[])
                esi.on_wait = [w]
                extra.ins.sync_info = esi
        self.nc.sync.drain()
        self.nc.all_engine_barrier()
        assert self.sems is not None
        popped = self.nc._tile_sem_poison_stack.pop()
        assert popped is self._sem_poison
        self.nc.clear_and_free_semaphores(list(self.sems.allocated().values()))
        self.nc.all_engine_barrier()


def _split_sync_waits(nc, cap=1):
    # This walrus build encodes at most one sync-wait per instruction.
    # Hoist excess waits onto same-engine NoOps placed just before the
    # instruction (engines are in-order, so this preserves semantics).
    for f in nc.m.functions:
        for bb in f.blocks:
            out = []
            changed = False
            for inst in bb.instructions:
                si = inst.sync_info
                waits = list(si.on_wait) if si else []
                if len(waits) > cap:
                    changed = True
                    for w in waits[:-cap]:
                        nop = mybir.InstNoOp(
                            name=nc.get_next_instruction_name(), ins=[], outs=[]
                        )
                        nop.engine = inst.engine
                        nop.sync_info = mybir.SyncInfo(on_wait=[w], on_update=[])
                        out.append(nop)
                    si.on_wait = waits[-cap:]
                    inst.sync_info = si
                out.append(inst)
            if changed:
                bb.instructions = out


def _build_program():
    nc = bass.Bass()
    ste_d = nc.dram_tensor("ste", [128, SEG], DT, kind="ExternalInput")
    sto_d = nc.dram_tensor("sto", [128, SEG], DT, kind="ExternalInput")
    wts_d = nc.dram_tensor("wts", [128, 64 * 128], DT, kind="ExternalInput")
    melt_d = nc.dram_tensor("melt", [128, 4 * 80], DT, kind="ExternalInput")
    out_d = nc.dram_tensor("out", [80, F], mybir.dt.float32, kind="ExternalOutput")

    f32 = mybir.dt.float32
    with _PatchedTileContext(nc) as tc:
        with (
            tc.tile_pool(name="const", bufs=1) as const,
            tc.tile_pool(name="work", bufs=2) as work,
            tc.tile_pool(name="psum", bufs=2, space="PSUM") as pp,
        ):
            # Each HWDGE engine (SP, ACT) has ONE FIFO dynamic queue; DMAs
            # complete in emission order per queue. Put the first-tile
            # critical set on SP (m0 weights in 4 separate tiles so the
            # first matmuls gate on 128KB) and the bulk on ACT's queue.
            ste_c, sto_c = [], []
            for t in range(N_TILES):
                c0 = t * NT
                c1 = min(SEG, (t + 1) * NT + 3)
                se = const.tile([128, c1 - c0], DT, tag=f"ste{t}")
                so = const.tile([128, c1 - c0], DT, tag=f"sto{t}")
                ste_c.append(se)
                sto_c.append(so)
            w0p = [
                const.tile([128, 512], DT, tag=f"w0p{j}", name=f"w0p{j}")
                for j in range(4)
            ]
            w_m = [None] + [
                const.tile([128, 2048], DT, tag=f"wm{m}", name=f"wm{m}")
                for m in (1, 2, 3)
            ]
            melt_sb = const.tile([128, 4 * 80], DT)

            nc.sync.dma_start(out=w0p[0][:], in_=wts_d[:, 0:512])
            nc.sync.dma_start(out=ste_c[0][:], in_=ste_d[:, 0 : NT + 3])
            nc.sync.dma_start(out=sto_c[0][:], in_=sto_d[:, 0 : NT + 3])
            for j in (1, 2, 3):
                nc.sync.dma_start(out=w0p[j][:], in_=wts_d[:, j * 512 : (j + 1) * 512])
            nc.sync.dma_start(out=melt_sb[:], in_=melt_d[:])
            for m in (1, 2, 3):
                nc.scalar.dma_start(
                    out=w_m[m][:], in_=wts_d[:, m * 2048 : (m + 1) * 2048]
                )
            for t in range(1, N_TILES):
                c0 = t * NT
                c1 = min(SEG, (t + 1) * NT + 3)
                nc.scalar.dma_start(out=ste_c[t][:], in_=ste_d[:, c0:c1])
                nc.scalar.dma_start(out=sto_c[t][:], in_=sto_d[:, c0:c1])
            outbuf = const.tile([80, F], f32)

            def lhsT(m, p, s):
                q = p * 2 + s
                if m == 0:
                    return w0p[q // 4][:, (q % 4) * 128 : (q % 4 + 1) * 128]
                return w_m[m][:, q * 128 : (q + 1) * 128]

            for t in range(N_TILES):
                t0 = t * NT
                mags = []
                for m in range(4):
                    rps = pp.tile([128, NT], f32, tag="R")
                    ips = pp.tile([128, NT], f32, tag="I")
                    for p in range(8):
                        st = ste_c[t] if p % 2 == 0 else sto_c[t]
                        rhs = st[:, p // 2 : p // 2 + NT]
                        nc.tensor.matmul(
                            rps[:], lhsT(m, p, 0), rhs,
                            start=(p == 0), stop=(p == 7),
                        )
                    for p in range(8):
                        st = ste_c[t] if p % 2 == 0 else sto_c[t]
                        rhs = st[:, p // 2 : p // 2 + NT]
                        nc.tensor.matmul(
                            ips[:], lhsT(m, p, 1), rhs,
                            start=(p == 0), stop=(p == 7),
                        )
                    a = work.tile([128, NT], f32, tag="a")
                    nc.scalar.square(a[:], rps[:])
                    b = work.tile([128, NT], f32, tag="b")
                    nc.scalar.square(b[:], ips[:])
                    m2 = work.tile([128, NT], f32, tag="m2")
                    nc.vector.tensor_tensor(m2[:], a[:], b[:], mybir.AluOpType.add)
                    mg = work.tile([128, NT], DT, tag=f"mag{m}")
                    nc.scalar.sqrt(mg[:], m2[:])
                    mags.append(mg)
                mel_ps = pp.tile([80, NT], f32, tag="mel")
                for m in range(4):
                    nc.tensor.matmul(
                        mel_ps[:], melt_sb[:, m * 80 : (m + 1) * 80], mags[m][:],
                        start=(m == 0), stop=(m == 3),
                    )
                nc.vector.tensor_copy(out=outbuf[:, t0 : t0 + NT], in_=mel_ps[:])

            # stream the log1p + output DMA in 4 column blocks so the
            # store overlaps the activation
            outln = const.tile([80, F], f32)
            for b0 in range(0, F, 1024):
                nc.scalar.activation(
                    outln[:, b0 : b0 + 1024], outbuf[:, b0 : b0 + 1024],
                    mybir.ActivationFunctionType.Ln, bias=1.0, scale=10000.0,
                )
                nc.sync.dma_start(
                    out=out_d[:, b0 : b0 + 1024], in_=outln[:, b0 : b0 + 1024]
                )
    _split_sync_waits(nc)
    return nc


def _prep_inputs(audio, basis_r, basis_i, mel_basis):
    audio = np.asarray(audio, dtype=np.float32)
    basis_r = np.asarray(basis_r, dtype=np.float32)
    basis_i = np.asarray(basis_i, dtype=np.float32)
    mel_basis = np.asarray(mel_basis, dtype=np.float32)

    # wts[:, ((m*8+p)*2+s)*128 + j] = basis_s[m*128+j, p*128:(p+1)*128]
    wts = np.empty((128, 64 * 128), dtype=NP_DT)
    for p in range(8):
        for m in range(4):
            for s, basis in enumerate((basis_r, basis_i)):
                q = ((m * 8 + p) * 2 + s) * 128
                wts[:, q : q + 128] = basis[
                    m * 128 : (m + 1) * 128, p * 128 : (p + 1) * 128
                ].T.astype(NP_DT)
    melt = np.empty((128, 4 * 80), dtype=NP_DT)
    for m in range(4):
        melt[:, m * 80 : (m + 1) * 80] = mel_basis[
            :, m * 128 : (m + 1) * 128
        ].T.astype(NP_DT)

    in_maps = []
    for b in range(N_CORES):
        row = audio[b]
        x = np.concatenate([row[PAD:0:-1], row, row[-2 : -PAD - 2 : -1]])
        xr = x.reshape(SEG, 256)
        ste = np.ascontiguousarray(xr[:, :128].T).astype(NP_DT)
        sto = np.ascontiguousarray(xr[:, 128:].T).astype(NP_DT)
        in_maps.append({"ste": ste, "sto": sto, "wts": wts, "melt": melt})
    return in_maps


def kernel(audio, basis_r, basis_i, mel_basis):
    if "nc" not in _cache:
        _cache["nc"] = _build_program()
    nc = _cache["nc"]
    in_maps = _prep_inputs(audio, basis_r, basis_i, mel_basis)

    trace = os.environ.get("MELSPEC_TRACE") == "1"
    if trace:
        import types
        import trn_agent_boot.trn_boot as tb
        import concourse.bass_utils as bu

        if "antenv.axon_hooks" not in sys.modules:
            hook = tb._ntff_profile_via_ctypes("/opt/axon/libaxon_pjrt.so")
            mod = types.ModuleType("antenv.axon_hooks")
            mod.get_axon_ntff_profile_hook = lambda: hook
            sys.modules["antenv.axon_hooks"] = mod
        bu.upload_artifacts = lambda tmpdir: f"local://{tmpdir}"

    res = run_bass_kernel_spmd(nc, in_maps, list(range(N_CORES)), trace=trace)
    _cache["last_results"] = res
    out = np.stack([res.results[i]["out"] for i in range(N_CORES)])
    return out.astype(np.float32)


# revision 15
# speedup vs baseline: 1.2582x; 1.1992x over previous
"""MelSpectrogram Trainium2 kernel.

Full inputs in, full output out. Data-parallel over batch B=8 across the
8 NeuronCores (one audio row per core); DFT basis and mel filterbank are
replicated (prepped host-side into matmul-friendly layouts).

Per-core device algorithm (one audio row, T=1048576):

The reflect-padded signal x (len 4099*256) is split host-side into even/
odd sample streams laid out as SBUF tensors SE/SO [128, 4099] with
SE[l,j] = x[2*(j*128+l)], SO[l,j] = x[2*(j*128+l)+1]. Because HOP=256
divides FILTER_LEN=1024, frame f's even-tap block p (taps 2*(p*128+l))
is SE[:, f+p] -- the windowed DFT needs no frame materialization, just
shifted rhs slices.

Radix-2 bin pairing: the DFT basis satisfies basis[512-k, n] =
+/-(-1)^n basis[k, n], so with P = Ce@xe, Q = Co@xo, V = Se@xe,
W = So@xo (even/odd tap split of the windowed cos/sin bases, bin rows
1..256):
    mag[k]      = sqrt((P+Q)^2 + (V+W)^2)   k = 1..256
    mag[512-k]  = sqrt((P-Q)^2 + (V-W)^2)   k = 1..255
Mel filter weights at bins 0 and 512 are exactly zero (fmin=0,
fmax=sr/2 edge filters), so bins 1..511 cover everything: the DFT
matmul work halves versus the direct form, in clean 2x128-row tiles
(the bin-256 duplicate on the B side is zeroed in the permuted mel
matrix host-side).

magnitudes feed a PSUM-accumulated mel projection on PE; final
log1p(1e4*x) = Ln(1e4*x + 1) on ACT, streamed against the output DMA.
"""

import os
import sys

sys.path.insert(0, "/opt/trn_rl_repo")

import numpy as np
import concourse.bass as bass
import concourse.mybir as mybir
import concourse.tile as tile
from concourse.bass_utils import run_bass_kernel_spmd
from concourse.vector_clock import ScopedClock

N_CORES = 8
T = 1048576
PAD = 384
SEG = 4099  # (T + 2*PAD) / 256
F = 4096  # output frames
NT = 512  # frames per tile
N_TILES = F // NT
DT = mybir.dt.float16
NP_DT = np.float16

_cache = {}


class _PatchedTileContext(tile.TileContext):
    # This walrus build rejects >1 sync-wait per instruction (and any on
    # the kernel-tail Drain): carry the global-clock waits on
    # one-NoOp-per-wait ahead of the drain instead.
    def _drain_and_barrier(self, tick_clock, wait_clock):
        nop_inst = self.nc.sync.nop(nofuse=True, hint="pre_drain_waits")
        wait_clock.add_sem_waits(
            nop_inst.ins, ScopedClock({None: tick_clock.global_clock})
        )
        waits = list(nop_inst.ins.sync_info.on_wait)
        if len(waits) > 1:
            si = nop_inst.ins.sync_info
            si.on_wait = waits[:1]
            nop_inst.ins.sync_info = si
            for w in waits[1:]:
                extra = self.nc.sync.nop(nofuse=True, hint="pre_drain_waits")
                esi = extra.ins.sync_info or mybir.SyncInfo(on_wait=[], on_update=[])
                esi.on_wait = [w]
                extra.ins.sync_info = esi
        self.nc.sync.drain()
        self.nc.all_engine_barrier()
        assert self.sems is not None
        popped = self.nc._tile_sem_poison_stack.pop()
        assert popped is self._sem_poison
        self.nc.clear_and_free_semaphores(list(self.sems.allocated().values()))
        self.nc.all_engine_barrier()


def _split_sync_waits(nc, cap=1):
    # Hoist excess sync-waits onto same-engine NoOps placed just before
    # the instruction (engines are in-order, so semantics preserved).
    for f in nc.m.functions:
        for bb in f.blocks:
            out = []
            changed = False
            for inst in bb.instructions:
                si = inst.sync_info
                waits = list(si.on_wait) if si else []
                if len(waits) > cap:
                    changed = True
                    for w in waits[:-cap]:
                        nop = mybir.InstNoOp(
                            name=nc.get_next_instruction_name(), ins=[], outs=[]
                        )
                        nop.engine = inst.engine
                        nop.sync_info = mybir.SyncInfo(on_wait=[w], on_update=[])
                        out.append(nop)
                    si.on_wait = waits[-cap:]
                    inst.sync_info = si
                out.append(inst)
            if changed:
                bb.instructions = out


def _build_program():
    nc = bass.Bass()
    se_d = nc.dram_tensor("se", [128, SEG], DT, kind="ExternalInput")
    so_d = nc.dram_tensor("so", [128, SEG], DT, kind="ExternalInput")
    # 32 blocks of [128,128]: q = m*16 + p*4 + x, x in (Ce,Co,Se,So)
    wts_d = nc.dram_tensor("wts", [128, 32 * 128], DT, kind="ExternalInput")
    melt_d = nc.dram_tensor("melt", [128, 4 * 80], DT, kind="ExternalInput")
    out_d = nc.dram_tensor("out", [80, F], mybir.dt.float32, kind="ExternalOutput")

    f32 = mybir.dt.float32
    with _PatchedTileContext(nc) as tc:
        with (
            tc.tile_pool(name="const", bufs=1) as const,
            tc.tile_pool(name="work", bufs=2) as work,
            tc.tile_pool(name="psum", bufs=2, space="PSUM") as pp,
        ):
            # Each HWDGE engine (SP, ACT) has ONE FIFO dynamic queue; DMAs
            # complete in emission order per queue. First-tile critical
            # set rides SP; bulk rides ACT's queue.
            se_c, so_c = [], []
            for t in range(N_TILES):
                c0 = t * NT
                c1 = min(SEG, (t + 1) * NT + 3)
                se = const.tile([128, c1 - c0], DT, tag=f"se{t}", name=f"se{t}")
                so = const.tile([128, c1 - c0], DT, tag=f"so{t}", name=f"so{t}")
                se_c.append(se)
                so_c.append(so)
            # m=0 weights split per p so the first matmuls gate on 128KB
            w0p = [
                const.tile([128, 512], DT, tag=f"w0p{j}", name=f"w0p{j}")
                for j in range(4)
            ]
            w_m1 = const.tile([128, 2048], DT, tag="wm1", name="wm1")
            melt_sb = const.tile([128, 4 * 80], DT)

            nc.sync.dma_start(out=w0p[0][:], in_=wts_d[:, 0:512])
            nc.sync.dma_start(out=se_c[0][:], in_=se_d[:, 0 : NT + 3])
            nc.sync.dma_start(out=so_c[0][:], in_=so_d[:, 0 : NT + 3])
            for j in (1, 2, 3):
                nc.sync.dma_start(out=w0p[j][:], in_=wts_d[:, j * 512 : (j + 1) * 512])
            nc.sync.dma_start(out=melt_sb[:], in_=melt_d[:])
            nc.scalar.dma_start(out=w_m1[:], in_=wts_d[:, 2048:4096])
            for t in range(1, N_TILES):
                c0 = t * NT
                c1 = min(SEG, (t + 1) * NT + 3)
                nc.scalar.dma_start(out=se_c[t][:], in_=se_d[:, c0:c1])
                nc.scalar.dma_start(out=so_c[t][:], in_=so_d[:, c0:c1])
            outbuf = const.tile([80, F], f32)

            def lhsT(m, p, x):
                if m == 0:
                    return w0p[p][:, x * 128 : (x + 1) * 128]
                return w_m1[:, (p * 4 + x) * 128 : (p * 4 + x + 1) * 128]

            BP = mybir.AluOpType.bypass
            ADD = mybir.AluOpType.add
            SUB = mybir.AluOpType.subtract
            MUL = mybir.AluOpType.mult

            for t in range(N_TILES):
                t0 = t * NT
                mags = {}
                for m in range(2):
                    P = pp.tile([128, NT], f32, tag="P")
                    Q = pp.tile([128, NT], f32, tag="Q")
                    V = pp.tile([128, NT], f32, tag="V")
                    W = pp.tile([128, NT], f32, tag="W", bufs=1)
                    for p in range(4):
                        rhsE = se_c[t][:, p : p + NT]
                        rhsO = so_c[t][:, p : p + NT]
                        st = dict(start=(p == 0), stop=(p == 3))
                        nc.tensor.matmul(P[:], lhsT(m, p, 0), rhsE, **st)
                        nc.tensor.matmul(Q[:], lhsT(m, p, 1), rhsO, **st)
                        nc.tensor.matmul(V[:], lhsT(m, p, 2), rhsE, **st)
                        nc.tensor.matmul(W[:], lhsT(m, p, 3), rhsO, **st)
                    qs = work.tile([128, NT], DT, tag="qs")
                    nc.scalar.copy(qs[:], Q[:])
                    ws = work.tile([128, NT], DT, tag="ws")
                    nc.scalar.copy(ws[:], W[:])
                    u = work.tile([128, NT], DT, tag="u")
                    nc.vector.scalar_tensor_tensor(u[:], P[:], 0.0, qs[:], BP, ADD)
                    u2 = work.tile([128, NT], DT, tag="u2")
                    nc.vector.scalar_tensor_tensor(u2[:], P[:], 0.0, qs[:], BP, SUB)
                    v = work.tile([128, NT], DT, tag="v")
                    nc.vector.scalar_tensor_tensor(v[:], V[:], 0.0, ws[:], BP, ADD)
                    v2 = work.tile([128, NT], DT, tag="v2")
                    nc.vector.scalar_tensor_tensor(v2[:], V[:], 0.0, ws[:], BP, SUB)
                    a = work.tile([128, NT], DT, tag="a")
                    nc.vector.tensor_tensor(a[:], u[:], u[:], MUL)
                    b = work.tile([128, NT], DT, tag="b")
                    nc.vector.tensor_tensor(b[:], v[:], v[:], MUL)
                    m2A = work.tile([128, NT], DT, tag="m2A")
                    nc.gpsimd.tensor_tensor(m2A[:], a[:], b[:], ADD)
                    magA = work.tile([128, NT], DT, tag=f"magA{m}")
                    nc.scalar.sqrt(magA[:], m2A[:])
                    a2 = work.tile([128, NT], DT, tag="a2")
                    nc.gpsimd.tensor_tensor(a2[:], u2[:], u2[:], MUL)
                    b2 = work.tile([128, NT], DT, tag="b2")
                    nc.gpsimd.tensor_tensor(b2[:], v2[:], v2[:], MUL)
                    m2B = work.tile([128, NT], DT, tag="m2B")
                    nc.gpsimd.tensor_tensor(m2B[:], a2[:], b2[:], ADD)
                    magB = work.tile([128, NT], DT, tag=f"magB{m}")
                    nc.scalar.sqrt(magB[:], m2B[:])
                    mags[("A", m)] = magA
                    mags[("B", m)] = magB

                mel_ps = pp.tile([80, NT], f32, tag="mel", bufs=1)
                order = [("A", 0), ("A", 1), ("B", 0), ("B", 1)]
                for i, key in enumerate(order):
                    nc.tensor.matmul(
                        mel_ps[:], melt_sb[:, i * 80 : (i + 1) * 80], mags[key][:],
                        start=(i == 0), stop=(i == 3),
                    )
                nc.vector.tensor_copy(out=outbuf[:, t0 : t0 + NT], in_=mel_ps[:])

            # stream the log1p + output DMA in 4 column blocks
            outln = const.tile([80, F], f32)
            for b0 in range(0, F, 1024):
                nc.scalar.activation(
                    outln[:, b0 : b0 + 1024], outbuf[:, b0 : b0 + 1024],
                    mybir.ActivationFunctionType.Ln, bias=1.0, scale=10000.0,
                )
                nc.sync.dma_start(
                    out=out_d[:, b0 : b0 + 1024], in_=outln[:, b0 : b0 + 1024]
                )
    _split_sync_waits(nc)
    return nc


def _prep_inputs(audio, basis_r, basis_i, mel_basis):
    audio = np.asarray(audio, dtype=np.float32)
    basis_r = np.asarray(basis_r, dtype=np.float32)
    basis_i = np.asarray(basis_i, dtype=np.float32)
    mel_basis = np.asarray(mel_basis, dtype=np.float32)

    Ce = basis_r[:257, 0::2]
    Co = basis_r[:257, 1::2]
    Se = basis_i[:257, 0::2]
    So = basis_i[:257, 1::2]
    mats = (Ce, Co, Se, So)
    wts = np.empty((128, 32 * 128), dtype=NP_DT)
    for m in range(2):
        for p in range(4):
            for x in range(4):
                q = (m * 16 + p * 4 + x) * 128
                blk = mats[x][1 + m * 128 : 1 + (m + 1) * 128, p * 128 : (p + 1) * 128]
                wts[:, q : q + 128] = blk.T.astype(NP_DT)

    # mel matrix, bins permuted to the device's mag row order:
    # A side rows = bins 1..256; B side row j = bin 511-j (row 255 is the
    # bin-256 duplicate -> zeroed)
    melA = mel_basis[:, 1:257]
    melB = mel_basis[:, [511 - j for j in range(256)]].copy()
    melB[:, 255] = 0.0
    melt = np.empty((128, 4 * 80), dtype=NP_DT)
    melt[:, 0:80] = melA[:, 0:128].T.astype(NP_DT)
    melt[:, 80:160] = melA[:, 128:256].T.astype(NP_DT)
    melt[:, 160:240] = melB[:, 0:128].T.astype(NP_DT)
    melt[:, 240:320] = melB[:, 128:256].T.astype(NP_DT)

    in_maps = []
    for b in range(N_CORES):
        row = audio[b]
        x = np.concatenate([row[PAD:0:-1], row, row[-2 : -PAD - 2 : -1]])
        se = np.ascontiguousarray(x[0::2].reshape(SEG, 128).T).astype(NP_DT)
        so = np.ascontiguousarray(x[1::2].reshape(SEG, 128).T).astype(NP_DT)
        in_maps.append({"se": se, "so": so, "wts": wts, "melt": melt})
    return in_maps


def kernel(audio, basis_r, basis_i, mel_basis):
    if "nc" not in _cache:
        _cache["nc"] = _build_program()
    nc = _cache["nc"]
    in_maps = _prep_inputs(audio, basis_r, basis_i, mel_basis)

    trace = os.environ.get("MELSPEC_TRACE") == "1"
    if trace:
        import types
        import trn_agent_boot.trn_boot as tb
        import concourse.bass_utils as bu

        if "antenv.axon_hooks" not in sys.modules:
            hook = tb._ntff_profile_via_ctypes("/opt/axon/libaxon_pjrt.so")
            mod = types.ModuleType("antenv.axon_hooks")
            mod.get_axon_ntff_profile_hook = lambda: hook
            sys.modules["antenv.axon_hooks"] = mod
        bu.upload_artifacts = lambda tmpdir: f"local://{tmpdir}"

    res = run_bass_kernel_spmd(nc, in_maps, list(range(N_CORES)), trace=trace)
    _cache["last_results"] = res
    out = np.stack([res.results[i]["out"] for i in range(N_CORES)])
    return out.astype(np.float32)
